# revision 16
# baseline (speedup 1.0000x reference)
"""Contrastive cosine loss (B=8192, D=2048) on 8 Trainium2 NeuronCores.

Strategy (data-parallel over the batch dim, 1024 rows/core):
  * The reference's hard-negative mining selects, per row, the candidate with
    the smaller cosine distance (ties -> later candidate) and then recomputes
    that distance.  Algebraically the recomputed negative distance is simply
    min(d0, d1)  ==  1 - max(cos0, cos1), so no index selection is needed:
    the loss only requires per-row dot products, squared norms, and a max.
  * Per core: dense-load its local img/text rows, dma_gather the (cross-shard)
    candidate rows from the full embeddings, compute
        dots   on DVE  via fused tensor_tensor_reduce (mult + add-accum)
        norms  on ACT  via activation(Square, accum_out=...)
    then a small [128, 8] epilogue producing per-row cosine and the two relu
    loss terms.  Host concatenates per-core outputs and takes the means.
"""

import os

import numpy as np

import concourse.bacc as bacc
import concourse.bass as bass
import concourse.mybir as mybir
import concourse.tile as tile
from concourse.bass_utils import run_bass_kernel_spmd

B, D = 8192, 2048
N_CORES = 8
ROWS = B // N_CORES          # rows per core
P = 128                      # partitions
NTILES = ROWS // P           # 8 row-tiles per core
MARGIN = 0.2
EPS = 1e-8

# "f32" or "bf16" (bf16 halves HBM traffic; inputs are cast on host)
DTYPE = os.environ.get("BASSK_DTYPE", "f32")
BUFS = int(os.environ.get("BASSK_BUFS", "2"))
# "amr" (fused DVE square+reduce) or "act" (ACT square + DVE reduce)
NORM_MODE = os.environ.get("BASSK_NORM", "amr")
# outer hardware-loop repetitions (>1 only for timing measurements)
OUTER = int(os.environ.get("BASSK_OUTER", "1"))

F32 = mybir.dt.float32
AF = mybir.ActivationFunctionType
OP = mybir.AluOpType


def build_program(dtype_name: str = DTYPE, outer: int = OUTER) -> bass.Bass:
    dt_in = F32 if dtype_name == "f32" else mybir.dt.bfloat16

    nc = bacc.Bacc(
        "TRN2",
        target_bir_lowering=False,
        debug=False,
        enable_asserts=True,
        num_devices=N_CORES,
    )

    img_full = nc.dram_tensor("img_full", [B, D], dt_in, kind="ExternalInput").ap()
    text_full = nc.dram_tensor("text_full", [B, D], dt_in, kind="ExternalInput").ap()
    img_loc = nc.dram_tensor("img_loc", [ROWS, D], dt_in, kind="ExternalInput").ap()
    text_loc = nc.dram_tensor("text_loc", [ROWS, D], dt_in, kind="ExternalInput").ap()
    # dma_gather index layout: index i of a 256-wide gather lives at
    # [i % 16, i // 16] of a [16, 16] block, replicated to 128 partitions.
    # Tile t's block occupies columns [16*t, 16*(t+1)).
    idx_img = nc.dram_tensor(
        "idx_img", [P, NTILES * 16], mybir.dt.int16, kind="ExternalInput"
    ).ap()
    idx_text = nc.dram_tensor(
        "idx_text", [P, NTILES * 16], mybir.dt.int16, kind="ExternalInput"
    ).ap()

    cos_out = nc.dram_tensor("cos_out", [P, NTILES], F32, kind="ExternalOutput").ap()
    i2t_out = nc.dram_tensor("i2t_out", [P, NTILES], F32, kind="ExternalOutput").ap()
    t2i_out = nc.dram_tensor("t2i_out", [P, NTILES], F32, kind="ExternalOutput").ap()

    with tile.TileContext(nc) as tc:
        with (
            tc.tile_pool(name="io", bufs=2) as io_pool,
            tc.tile_pool(name="idx", bufs=1) as idx_pool,
            tc.tile_pool(name="scr", bufs=2) as scr_pool,
            tc.tile_pool(name="stat", bufs=1) as stat_pool,
            tc.tile_pool(name="ep", bufs=1) as ep_pool,
        ):
          def body():
            idx_i = idx_pool.tile([P, NTILES * 16], mybir.dt.int16, tag="idx_i")
            nc.sync.dma_start(out=idx_i[:], in_=idx_img)
            idx_t = idx_pool.tile([P, NTILES * 16], mybir.dt.int16, tag="idx_t")
            nc.sync.dma_start(out=idx_t[:], in_=idx_text)

            def stat(tag):
                return stat_pool.tile([P, NTILES], F32, tag=tag, name=tag)

            s_pos = stat("s_pos")            # <img_i, text_i>
            wg0, wg1 = stat("wg0"), stat("wg1")  # <text_i, img[cand_img]>
            wh0, wh1 = stat("wh0"), stat("wh1")  # <img_i, text[cand_text]>
            sii, stt = stat("sii"), stat("stt")  # |img_i|^2, |text_i|^2
            sg0, sg1 = stat("sg0"), stat("sg1")  # |img[cand]|^2
            sh0, sh1 = stat("sh0"), stat("sh1")  # |text[cand]|^2

            for t in range(NTILES):
                li = io_pool.tile([P, D], dt_in, tag="li")
                nc.sync.dma_start(out=li[:], in_=img_loc[t * P : (t + 1) * P, :])
                lt = io_pool.tile([P, D], dt_in, tag="lt")
                nc.sync.dma_start(out=lt[:], in_=text_loc[t * P : (t + 1) * P, :])
                g = io_pool.tile([P, 2, D], dt_in, tag="g")
                nc.gpsimd.dma_gather(
                    g[:], img_full, idx_i[:, t * 16 : (t + 1) * 16], 2 * P, 2 * P, D
                )
                h = io_pool.tile([P, 2, D], dt_in, tag="h")
                nc.gpsimd.dma_gather(
                    h[:], text_full, idx_t[:, t * 16 : (t + 1) * 16], 2 * P, 2 * P, D
                )

                # norms: NORM_MODE "amr" = fused square+reduce on DVE;
                # "act" = ACT square (full tile) then DVE reduce_sum.
                for src, st in (
                    (li[:], sii),
                    (lt[:], stt),
                    (g[:, 0, :], sg0),
                    (g[:, 1, :], sg1),
                    (h[:, 0, :], sh0),
                    (h[:, 1, :], sh1),
                ):
                    if NORM_MODE == "amr":
                        vscr = scr_pool.tile([P, D], dt_in, tag="vscr", name="vscr")
                        nc.vector.affine_mul_reduce(
                            out=vscr[:], accum_out=st[:, t : t + 1],
                            in0=src, in1=src, scale=1.0, bias=0.0,
                        )
                    else:
                        ascr = scr_pool.tile([P, D], dt_in, tag="ascr", name="ascr")
                        nc.scalar.activation(ascr[:], src, AF.Square)
                        nc.vector.reduce_sum(
                            st[:, t : t + 1], ascr[:], axis=mybir.AxisListType.X
                        )

                # dots: fused multiply + add-reduce on DVE (custom op)
                for a, b_, st in (
                    (li[:], lt[:], s_pos),
                    (lt[:], g[:, 0, :], wg0),
                    (lt[:], g[:, 1, :], wg1),
                    (li[:], h[:, 0, :], wh0),
                    (li[:], h[:, 1, :], wh1),
                ):
                    vscr = scr_pool.tile([P, D], dt_in, tag="vscr", name="vscr")
                    nc.vector.affine_mul_reduce(
                        out=vscr[:], accum_out=st[:, t : t + 1],
                        in0=a, in1=b_, scale=1.0, bias=0.0,
                    )

            # ---- epilogue on [P, NTILES] f32 tiles ----
            ep_n = [0]

            def ep():
                ep_n[0] += 1
                return ep_pool.tile([P, NTILES], F32, tag=f"ep{ep_n[0]}", name=f"ep{ep_n[0]}")

            def cos_of(w, sa, sb):
                # w / max(sqrt(sa * sb), EPS)
                n2 = ep()
                nc.vector.tensor_mul(out=n2[:], in0=sa[:], in1=sb[:])
                n = ep()
                nc.scalar.sqrt(n[:], n2[:])
                nc.vector.tensor_scalar_max(out=n[:], in0=n[:], scalar1=EPS)
                r = ep()
                nc.vector.reciprocal(out=r[:], in_=n[:])
                c = ep()
                nc.vector.tensor_mul(out=c[:], in0=w[:], in1=r[:])
                return c

            cpos = cos_of(s_pos, sii, stt)
            cg0 = cos_of(wg0, stt, sg0)
            cg1 = cos_of(wg1, stt, sg1)
            cg = ep()
            nc.vector.tensor_max(out=cg[:], in0=cg0[:], in1=cg1[:])
            ch0 = cos_of(wh0, sii, sh0)
            ch1 = cos_of(wh1, sii, sh1)
            ch = ep()
            nc.vector.tensor_max(out=ch[:], in0=ch0[:], in1=ch1[:])

            # i2t loss: relu(ch - cpos + MARGIN); t2i: relu(cg - cpos + MARGIN)
            di = ep()
            nc.vector.tensor_sub(out=di[:], in0=ch[:], in1=cpos[:])
            li_t = ep()
            nc.vector.tensor_scalar(
                out=li_t[:], in0=di[:], scalar1=MARGIN, scalar2=0.0,
                op0=OP.add, op1=OP.max,
            )
            dt_ = ep()
            nc.vector.tensor_sub(out=dt_[:], in0=cg[:], in1=cpos[:])
            lt_t = ep()
            nc.vector.tensor_scalar(
                out=lt_t[:], in0=dt_[:], scalar1=MARGIN, scalar2=0.0,
                op0=OP.add, op1=OP.max,
            )

            nc.sync.dma_start(out=cos_out, in_=cpos[:])
            nc.sync.dma_start(out=i2t_out, in_=li_t[:])
            nc.sync.dma_start(out=t2i_out, in_=lt_t[:])

          if outer == 1:
            body()
          else:
            with tc.For_i(0, outer, 1):
                body()

    nc.compile()
    return nc


def build_program_pe(outer: int = OUTER) -> bass.Bass:
    """bf16 variant with transposed tiles (D on partitions) so ALL 11
    per-row reductions run on the TensorEngine as ones-vector matmuls
    accumulating into one PSUM tile; DVE/ACT only do the elementwise
    products.  Loads (locals included) use dma_gather(transpose=True).

    Stats PSUM layout: [NTILES, 11*P] f32; stat s of tile t lives at
    [t, s*P:(s+1)*P].  Stat order: spos wg0 wg1 wh0 wh1 sii stt sg0 sg1
    sh0 sh1.
    """
    from contextlib import ExitStack

    BF = mybir.dt.bfloat16
    C = D // P  # 16 chunks

    nc = bacc.Bacc(
        "TRN2",
        target_bir_lowering=False,
        debug=False,
        enable_asserts=True,
        num_devices=N_CORES,
    )

    img_full = nc.dram_tensor("img_full", [B, D], BF, kind="ExternalInput").ap()
    text_full = nc.dram_tensor("text_full", [B, D], BF, kind="ExternalInput").ap()
    idx_img = nc.dram_tensor(
        "idx_img", [P, NTILES * 16], mybir.dt.int16, kind="ExternalInput"
    ).ap()
    idx_text = nc.dram_tensor(
        "idx_text", [P, NTILES * 16], mybir.dt.int16, kind="ExternalInput"
    ).ap()
    idx_loc = nc.dram_tensor(
        "idx_loc", [P, NTILES * 8], mybir.dt.int16, kind="ExternalInput"
    ).ap()

    cos_out = nc.dram_tensor("cos_out", [NTILES, P], F32, kind="ExternalOutput").ap()
    i2t_out = nc.dram_tensor("i2t_out", [NTILES, P], F32, kind="ExternalOutput").ap()
    t2i_out = nc.dram_tensor("t2i_out", [NTILES, P], F32, kind="ExternalOutput").ap()

    with tile.TileContext(nc) as tc, ExitStack() as mmctx:
        with (
            tc.tile_pool(name="io", bufs=BUFS) as io_pool,
            tc.tile_pool(name="idx", bufs=1) as idx_pool,
            tc.tile_pool(name="prod", bufs=3) as prod_pool,
            tc.tile_pool(name="psum", bufs=1, space="PSUM") as psum_pool,
            tc.tile_pool(name="ep", bufs=1) as ep_pool,
        ):
            def body():
                ii = idx_pool.tile([P, NTILES * 16], mybir.dt.int16, tag="ii", name="ii")
                nc.sync.dma_start(out=ii[:], in_=idx_img)
                it = idx_pool.tile([P, NTILES * 16], mybir.dt.int16, tag="it", name="it")
                nc.sync.dma_start(out=it[:], in_=idx_text)
                il = idx_pool.tile([P, NTILES * 8], mybir.dt.int16, tag="il", name="il")
                nc.sync.dma_start(out=il[:], in_=idx_loc)
                ones = idx_pool.tile([P, 1], BF, tag="ones", name="ones")
                nc.vector.memset(ones[:], 1.0)

                ps = psum_pool.tile([NTILES, 11 * P], F32, tag="ps", name="ps")

                for t in range(NTILES):
                    lit = io_pool.tile([P, C, P], BF, tag="lit", name="lit")
                    nc.gpsimd.dma_gather(
                        lit[:], img_full, il[:, t * 8 : (t + 1) * 8], P, P, D,
                        transpose=True,
                    )
                    ltt = io_pool.tile([P, C, P], BF, tag="ltt", name="ltt")
                    nc.gpsimd.dma_gather(
                        ltt[:], text_full, il[:, t * 8 : (t + 1) * 8], P, P, D,
                        transpose=True,
                    )
                    gt = io_pool.tile([P, C, 2 * P], BF, tag="gt", name="gt")
                    nc.gpsimd.dma_gather(
                        gt[:], img_full, ii[:, t * 16 : (t + 1) * 16], 2 * P, 2 * P, D,
                        transpose=True,
                    )
                    ht = io_pool.tile([P, C, 2 * P], BF, tag="ht", name="ht")
                    nc.gpsimd.dma_gather(
                        ht[:], text_full, it[:, t * 16 : (t + 1) * 16], 2 * P, 2 * P, D,
                        transpose=True,
                    )

                    g0, g1 = gt[:, :, 0:P], gt[:, :, P : 2 * P]
                    h0, h1 = ht[:, :, 0:P], ht[:, :, P : 2 * P]
                    # (operand0, operand1 or None for ACT square, engine)
                    sets = (
                        (lit[:], ltt[:], "dve"),   # 0 spos
                        (ltt[:], g0, "dve"),       # 1 wg0
                        (ltt[:], g1, "dve"),       # 2 wg1
                        (lit[:], h0, "dve"),       # 3 wh0
                        (lit[:], h1, "dve"),       # 4 wh1
                        (lit[:], None, "act"),     # 5 sii
                        (ltt[:], None, "act"),     # 6 stt
                        (g0, g0, "dve"),           # 7 sg0
                        (g1, g1, "dve"),           # 8 sg1
                        (h0, None, "act"),         # 9 sh0
                        (h1, h1, "dve"),           # 10 sh1
                    )
                    for s, (a, b_, eng) in enumerate(sets):
                        prod = prod_pool.tile([P, C, P], BF, tag="prod", name="prod")
                        if eng == "act":
                            nc.scalar.activation(prod[:], a, AF.Square)
                        else:
                            nc.vector.tensor_mul(out=prod[:], in0=a, in1=b_ if b_ is not None else a)
                        for c in range(C):
                            nc.tensor.matmul(
                                mmctx,
                                ps[t : t + 1, s * P : (s + 1) * P],
                                ones[:],
                                prod[:, c, :],
                                start=(c == 0),
                                stop=(c == C - 1),
                            )

                # ---- epilogue on [NTILES, P] f32 ----
                st = ep_pool.tile([NTILES, 11 * P], F32, tag="st", name="st")
                nc.vector.tensor_copy(out=st[:], in_=ps[:])

                def sl(s):
                    return st[:, s * P : (s + 1) * P]

                ep_n = [0]

                def ep():
                    ep_n[0] += 1
                    return ep_pool.tile(
                        [NTILES, P], F32, tag=f"ep{ep_n[0]}", name=f"ep{ep_n[0]}"
                    )

                def cos_of(w, sa, sb):
                    n2 = ep()
                    nc.vector.tensor_mul(out=n2[:], in0=sa, in1=sb)
                    n = ep()
                    nc.scalar.sqrt(n[:], n2[:])
                    nc.vector.tensor_scalar_max(out=n[:], in0=n[:], scalar1=EPS)
                    r = ep()
                    nc.vector.reciprocal(out=r[:], in_=n[:])
                    c = ep()
                    nc.vector.tensor_mul(out=c[:], in0=w, in1=r[:])
                    return c

                cpos = cos_of(sl(0), sl(5), sl(6))
                cg0 = cos_of(sl(1), sl(6), sl(7))
                cg1 = cos_of(sl(2), sl(6), sl(8))
                cg = ep()
                nc.vector.tensor_max(out=cg[:], in0=cg0[:], in1=cg1[:])
                ch0 = cos_of(sl(3), sl(5), sl(9))
                ch1 = cos_of(sl(4), sl(5), sl(10))
                ch = ep()
                nc.vector.tensor_max(out=ch[:], in0=ch0[:], in1=ch1[:])

                di = ep()
                nc.vector.tensor_sub(out=di[:], in0=ch[:], in1=cpos[:])
                li_t = ep()
                nc.vector.tensor_scalar(
                    out=li_t[:], in0=di[:], scalar1=MARGIN, scalar2=0.0,
                    op0=OP.add, op1=OP.max,
                )
                dt_ = ep()
                nc.vector.tensor_sub(out=dt_[:], in0=cg[:], in1=cpos[:])
                lt_t = ep()
                nc.vector.tensor_scalar(
                    out=lt_t[:], in0=dt_[:], scalar1=MARGIN, scalar2=0.0,
                    op0=OP.add, op1=OP.max,
                )

                nc.sync.dma_start(out=cos_out, in_=cpos[:])
                nc.sync.dma_start(out=i2t_out, in_=li_t[:])
                nc.sync.dma_start(out=t2i_out, in_=lt_t[:])

            if outer == 1:
                body()
            else:
                with tc.For_i(0, outer, 1):
                    body()

    nc.compile()
    return nc


def _wrap_idx(cand: np.ndarray) -> np.ndarray:
    """[ROWS, 2] int candidate ids -> [128, NTILES*16] int16 dma_gather layout.

    For tile t the 256 gathered indices are ordered [cand0(rows), cand1(rows)];
    index i sits at [i % 16, t*16 + i // 16], replicated across partitions.
    """
    u = np.concatenate(
        [cand[:, 0].reshape(NTILES, P), cand[:, 1].reshape(NTILES, P)], axis=1
    )  # [t, 256]
    blocks = u.reshape(NTILES, 16, 16).transpose(0, 2, 1)  # [t, p16, col]
    arr16 = blocks.transpose(1, 0, 2).reshape(16, NTILES * 16)
    return np.tile(arr16, (P // 16, 1)).astype(np.int16)


def _wrap_flat(ids: np.ndarray, per_tile: int) -> np.ndarray:
    """Flat per-core gather ids (NTILES*per_tile) -> [128, NTILES*per_tile//16]
    int16 dma_gather layout (per-tile 16-row blocks, replicated to 128)."""
    cols = per_tile // 16
    out = np.zeros((16, NTILES * cols), np.int16)
    for t in range(NTILES):
        u = ids[t * per_tile : (t + 1) * per_tile]
        out[:, t * cols : (t + 1) * cols] = u.reshape(cols, 16).T
    return np.tile(out, (P // 16, 1)).astype(np.int16)


_PROG = {}
LAST_RESULTS = None
LAST_IN_MAPS = None


def _get_program(dtype_name: str, outer: int = 1) -> bass.Bass:
    key = (dtype_name, outer)
    if key not in _PROG:
        if dtype_name == "pe":
            _PROG[key] = build_program_pe(outer)
        else:
            _PROG[key] = build_program(dtype_name, outer)
    return _PROG[key]


def kernel(**inputs) -> tuple:
    img = np.ascontiguousarray(np.asarray(inputs["img_embedding"], dtype=np.float32))
    text = np.ascontiguousarray(np.asarray(inputs["text_embedding"], dtype=np.float32))
    cand_img = np.asarray(inputs["cand_img"])
    cand_text = np.asarray(inputs["cand_text"])

    if DTYPE in ("bf16", "pe"):
        import ml_dtypes

        img_d = img.astype(ml_dtypes.bfloat16)
        text_d = text.astype(ml_dtypes.bfloat16)
    else:
        img_d, text_d = img, text

    nc = _get_program(DTYPE, OUTER)

    in_maps = []
    for c in range(N_CORES):
        sl = slice(c * ROWS, (c + 1) * ROWS)
        if DTYPE == "pe":
            ci = cand_img[sl]
            ct = cand_text[sl]
            flat_i = np.concatenate(
                [ci[:, 0].reshape(NTILES, P), ci[:, 1].reshape(NTILES, P)], axis=1
            ).reshape(-1)
            flat_t = np.concatenate(
                [ct[:, 0].reshape(NTILES, P), ct[:, 1].reshape(NTILES, P)], axis=1
            ).reshape(-1)
            in_maps.append(
                {
                    "img_full": img_d,
                    "text_full": text_d,
                    "idx_img": _wrap_flat(flat_i.astype(np.int64), 2 * P),
                    "idx_text": _wrap_flat(flat_t.astype(np.int64), 2 * P),
                    "idx_loc": _wrap_flat(
                        np.arange(c * ROWS, (c + 1) * ROWS, dtype=np.int64), P
                    ),
                }
            )
        else:
            in_maps.append(
                {
                    "img_full": img_d,
                    "text_full": text_d,
                    "img_loc": np.ascontiguousarray(img_d[sl]),
                    "text_loc": np.ascontiguousarray(text_d[sl]),
                    "idx_img": _wrap_idx(cand_img[sl]),
                    "idx_text": _wrap_idx(cand_text[sl]),
                }
            )

    trace = bool(int(os.environ.get("BASSK_TRACE", "0")))
    res = run_bass_kernel_spmd(
        nc, in_maps, core_ids=list(range(N_CORES)), trace=trace
    )
    global LAST_RESULTS, LAST_IN_MAPS
    LAST_RESULTS = res
    LAST_IN_MAPS = in_maps

    cos_parts, i2t_parts, t2i_parts = [], [], []
    for c in range(N_CORES):
        out = res.results[c]
        if DTYPE == "pe":
            # [NTILES, 128]: row r = t*128 + p at [t, p] -> flatten directly
            unpack = lambda a: np.asarray(a).reshape(-1)
        else:
            # [128, NTILES] with row r = t*128 + p at [p, t] -> transpose+flatten
            unpack = lambda a: np.asarray(a).T.reshape(-1)
        cos_parts.append(unpack(out["cos_out"]))
        i2t_parts.append(unpack(out["i2t_out"]))
        t2i_parts.append(unpack(out["t2i_out"]))

    cos = np.concatenate(cos_parts).astype(np.float32)
    i2t = np.concatenate(i2t_parts)
    t2i = np.concatenate(t2i_parts)
    loss = np.float32(
        i2t.mean(dtype=np.float64) + t2i.mean(dtype=np.float64)
    )
    return np.asarray(loss, dtype=np.float32), cos, cos.copy()
